# revision 1
# baseline (speedup 1.0000x reference)
"""NNUE HalfKP embedding-bag kernel, v2: fp8 pair-cell gather + DoubleRow matmuls.

The v1 kernel is bottlenecked by gpsimd dma_gather descriptor generation
(~7.8 ns per index, serial on the Q7 pair). v2 attacks the index count:

  * Table stored as PAIR CELLS: T2[c] = [T[2c]; T[2c+1]] in fp8 -> one gather
    index fetches TWO vocab rows (1 KB). Lookups are deduplicated at cell
    granularity per 128-row batch chunk ACROSS both stm+nstm sets:
    8192 lookups -> ~6750 distinct cells (-25% descriptors).
  * Cell index < 20481 fits int16 -> single gather per chunk (no low/high
    vocab split).
  * Routing/weighting on the PE with fp8 DoubleRow matmuls: each slot holds
    2 rows (reduction tile 2 per partition), lhsT [128, 2, 128] carries
    per-(row, batch) weights. Cells ordered [stm-only | shared | nstm-only]
    so the stm PSUM only consumes runs [0, S_RUNS) and the nstm PSUM only
    runs [N_START, NRUNS) (static windows, host-side placement asserts).
  * Each chunk gather is split into two sub-gathers (28 + 27 runs) to stay
    under the 256-descriptor/engine SWDGE ring carveout; pad slots gather
    BIAS_CELL (the trailing--1 trim path wedges the device).
  * Bucket selection folded host-side into w_sel/ob_sel as in v1.
"""

import sys

sys.path.insert(0, "/opt/trn_rl_repo")

import numpy as np
import ml_dtypes

import concourse.bass as bass
import concourse.mybir as mybir
from concourse import bacc
from concourse.tile import TileContext
from concourse.bass_utils import run_bass_kernel_spmd

FP8 = ml_dtypes.float8_e4m3
BF16 = ml_dtypes.bfloat16

B = 8192
K = 32
F = 512
FT_VOCAB = 40960
FFT_VOCAB = 640
N_CORES = 8
BC = B // N_CORES          # rows per core = 1024
CH = BC // 128             # chunks per core = 8

CELLS = FT_VOCAB // 2      # 20480 pair cells
BIAS_CELL = CELLS          # cell 20480 = [bias_row; zeros]
CAP = 7040                 # gather slot capacity per chunk (55 runs)
NRUNS = CAP // 128         # 55
S_RUNS = 32                # stm PSUM consumes runs [0, 32)
N_START = 22               # nstm PSUM consumes runs [22, 55)
N_RUNS = NRUNS - N_START   # 33
SO_MIN = N_START * 128     # stm-only region padded to >= 2816
SREF_MAX = S_RUNS * 128    # stm-referenced cells must end <= 4096
NIDX16 = CAP // 16         # 448 int16 idx columns
# The SWDGE descriptor ring carveout is 16 KB/partition -> 256 descriptors per
# DMA-engine ring; one gather may generate at most ~16*255 descriptors, so the
# 7040-slot chunk gather is split into two sub-gathers (28 + 27 runs).
GSPLIT = 3584              # 28 runs, 225 descs/engine < 256

GATH_BUFS = 2

_compiled = None


def _build():
    nc = bacc.Bacc("TRN2", target_bir_lowering=False, debug=False, num_devices=N_CORES)

    t2_d = nc.dram_tensor("t2", [CELLS + 1, 2 * F], mybir.dt.float8e4, kind="ExternalInput")
    idx_d = nc.dram_tensor("idx", [CH, 128, NIDX16], mybir.dt.int16, kind="ExternalInput")
    ws_d = nc.dram_tensor("w_stm", [CH, 128, S_RUNS * 256], mybir.dt.float8e4, kind="ExternalInput")
    wn_d = nc.dram_tensor("w_nstm", [CH, 128, N_RUNS * 256], mybir.dt.float8e4, kind="ExternalInput")
    wsel_d = nc.dram_tensor("w_sel", [CH, 128, 2 * F], mybir.dt.bfloat16, kind="ExternalInput")
    obsel_d = nc.dram_tensor("ob_sel", [CH, 128, 1], mybir.dt.float32, kind="ExternalInput")
    out_d = nc.dram_tensor("out", [BC, 1], mybir.dt.float32, kind="ExternalOutput")

    with TileContext(nc) as tc:
        with (
            tc.tile_pool(name="idx", bufs=CH) as idxp,
            tc.tile_pool(name="gath", bufs=GATH_BUFS) as gathp,
            tc.tile_pool(name="wblk", bufs=2) as wblkp,
            tc.tile_pool(name="psum", bufs=4, space="PSUM") as psump,
            tc.tile_pool(name="hid", bufs=2) as hidp,
            tc.tile_pool(name="wsel", bufs=2) as wselp,
            tc.tile_pool(name="fin", bufs=4) as finp,
        ):
            # prefetch every chunk's idx tile up front so gathers never wait
            # behind the Sync engine's per-chunk DMA queue
            idxts = []
            for ch in range(CH):
                idxt = idxp.tile([128, NIDX16], mybir.dt.int16, tag="idx")
                nc.sync.dma_start(out=idxt[:], in_=idx_d[ch])
                idxts.append(idxt)

            for ch in range(CH):
                idxt = idxts[ch]
                rt = gathp.tile([128, NRUNS * 2 * F], mybir.dt.float8e4, tag="gath")
                # sub-gather 1: positions [0, GSPLIT) — never trimmed
                nc.gpsimd.dma_gather(
                    out_ap=rt[:, : GSPLIT // 128 * 2 * F].rearrange(
                        "p (s e) -> p s e", e=2 * F
                    ),
                    in_ap=t2_d[:, :],
                    idxs_ap=idxt[:, : GSPLIT // 16],
                    num_idxs=GSPLIT,
                    num_idxs_reg=GSPLIT,
                    elem_size=2 * F,
                    single_packet=False,
                )
                # sub-gather 2: positions [GSPLIT, CAP). No trailing-negative
                # trim: the trim path (value_load num_idxs_reg + -1 indices)
                # reproducibly wedges the device, so pads are valid BIAS_CELL
                # indices gathered at full count.
                nc.gpsimd.dma_gather(
                    out_ap=rt[:, GSPLIT // 128 * 2 * F :].rearrange(
                        "p (s e) -> p s e", e=2 * F
                    ),
                    in_ap=t2_d[:, :],
                    idxs_ap=idxt[:, GSPLIT // 16 :],
                    num_idxs=CAP - GSPLIT,
                    num_idxs_reg=CAP - GSPLIT,
                    elem_size=2 * F,
                    single_packet=False,
                )

                # W blocks ride the Scalar engine's HWDGE queue so they never
                # queue behind the Sync engine's output-stage DMAs
                wst = wblkp.tile([128, S_RUNS * 256], mybir.dt.float8e4, tag="ws")
                nc.scalar.dma_start(out=wst[:], in_=ws_d[ch])
                wnt = wblkp.tile([128, N_RUNS * 256], mybir.dt.float8e4, tag="wn")
                nc.scalar.dma_start(out=wnt[:], in_=wn_d[ch])

                hid = hidp.tile([128, 2 * F], mybir.dt.bfloat16)

                ps_s = psump.tile([128, F], mybir.dt.float32, tag="ps_s")
                for q in range(S_RUNS):
                    nc.tensor.matmul(
                        out=ps_s[:],
                        lhsT=wst[:, q * 256 : (q + 1) * 256].rearrange(
                            "p (i m) -> p i m", i=2
                        ),
                        rhs=rt[:, q * 2 * F : (q + 1) * 2 * F].rearrange(
                            "p (i n) -> p i n", i=2
                        ),
                        start=(q == 0),
                        stop=(q == S_RUNS - 1),
                        perf_mode=mybir.MatmulPerfMode.DoubleRow,
                    )
                nc.vector.tensor_scalar(
                    out=hid[:, :F],
                    in0=ps_s[:],
                    scalar1=0.0,
                    scalar2=1.0,
                    op0=mybir.AluOpType.max,
                    op1=mybir.AluOpType.min,
                )

                ps_n = psump.tile([128, F], mybir.dt.float32, tag="ps_n")
                for j in range(N_RUNS):
                    q = N_START + j
                    nc.tensor.matmul(
                        out=ps_n[:],
                        lhsT=wnt[:, j * 256 : (j + 1) * 256].rearrange(
                            "p (i m) -> p i m", i=2
                        ),
                        rhs=rt[:, q * 2 * F : (q + 1) * 2 * F].rearrange(
                            "p (i n) -> p i n", i=2
                        ),
                        start=(j == 0),
                        stop=(j == N_RUNS - 1),
                        perf_mode=mybir.MatmulPerfMode.DoubleRow,
                    )
                nc.vector.tensor_scalar(
                    out=hid[:, F:],
                    in0=ps_n[:],
                    scalar1=0.0,
                    scalar2=1.0,
                    op0=mybir.AluOpType.max,
                    op1=mybir.AluOpType.min,
                )

                wsel = wselp.tile([128, 2 * F], mybir.dt.bfloat16)
                nc.sync.dma_start(out=wsel[:], in_=wsel_d[ch])
                obsel = finp.tile([128, 1], mybir.dt.float32, tag="ob")
                nc.sync.dma_start(out=obsel[:], in_=obsel_d[ch])

                prod = finp.tile([128, 2 * F], mybir.dt.float32, tag="prod")
                nc.vector.tensor_tensor(
                    out=prod[:], in0=hid[:], in1=wsel[:], op=mybir.AluOpType.mult
                )
                acc = finp.tile([128, 1], mybir.dt.float32, tag="acc")
                nc.vector.tensor_reduce(
                    out=acc[:], in_=prod[:], axis=mybir.AxisListType.X, op=mybir.AluOpType.add
                )
                sig = finp.tile([128, 1], mybir.dt.float32, tag="sig")
                nc.scalar.activation(
                    out=sig[:],
                    in_=acc[:],
                    func=mybir.ActivationFunctionType.Sigmoid,
                    bias=obsel[:],
                )
                nc.sync.dma_start(out=out_d[ch * 128 : (ch + 1) * 128, :], in_=sig[:])

    nc.compile()
    return nc


def _get_compiled():
    global _compiled
    if _compiled is None:
        _compiled = _build()
    return _compiled


def _wrap16(lst):
    """int16 index list -> [128, len/16] wrapped (i -> [i%16, i//16]) + replicated."""
    n = lst.shape[0]
    w = lst.reshape(n // 16, 16).T.astype(np.int16)     # [16, n/16]
    return np.tile(w, (8, 1))                            # [128, n/16]


def _prep_core(core, T2, values, stm, nstm, wsel_all, obsel_all):
    rows = slice(core * BC, (core + 1) * BC)
    v_core = np.asarray(values[rows], dtype=np.float32)
    stm_c = np.asarray(stm[rows], dtype=np.int64)
    nstm_c = np.asarray(nstm[rows], dtype=np.int64)

    idx16 = np.zeros((CH, 128, NIDX16), np.int16)
    Ws = np.zeros((CH, 128, S_RUNS, 2, 128), np.float32)
    Wn = np.zeros((CH, 128, N_RUNS, 2, 128), np.float32)

    m_of = np.repeat(np.arange(128), K)                  # flat lookup -> batch row

    for ch in range(CH):
        sl = slice(ch * 128, (ch + 1) * 128)
        val = v_core[sl].reshape(-1)
        cs = (stm_c[sl].reshape(-1) // 2)
        rs = (stm_c[sl].reshape(-1) % 2)
        cn = (nstm_c[sl].reshape(-1) // 2)
        rn = (nstm_c[sl].reshape(-1) % 2)

        u_s = np.unique(cs)
        u_n = np.unique(cn)
        shared = np.intersect1d(u_s, u_n, assume_unique=True)
        s_only = np.setdiff1d(u_s, shared, assume_unique=True)
        n_only = np.setdiff1d(u_n, shared, assume_unique=True)

        n_so = len(s_only)
        pad_so = max(0, SO_MIN - n_so)                   # keep nstm window start static
        layout = np.concatenate([
            s_only,
            # dummy pad: BIAS_CELL can never alias a real lookup (cells from
            # real indices are < CELLS), so pos_of stays unambiguous
            np.full(pad_so, BIAS_CELL, np.int64),
            shared,
            np.array([BIAS_CELL], np.int64),
            n_only,
        ])
        U = len(layout)
        sref_end = n_so + pad_so + len(shared) + 1
        assert sref_end <= SREF_MAX, (n_so, pad_so, len(shared))
        assert U <= CAP, U

        pos_of = np.empty(CELLS + 1, np.int32)
        # duplicates from pad cells are fine: only the FIRST occurrence
        # position is referenced by weights (assign in reverse so first wins)
        pos_of[layout[::-1]] = np.arange(U - 1, -1, -1, dtype=np.int32)

        ilist = np.full(CAP, BIAS_CELL, np.int16)   # tail pad: gathered, weight 0
        ilist[:U] = layout
        assert U > GSPLIT, U
        idx16[ch] = _wrap16(ilist)

        # stm weights -> Ws (runs [0, S_RUNS))
        p = pos_of[cs]
        assert p.max() < SREF_MAX
        np.add.at(Ws[ch], (p % 128, p // 128, rs, m_of), val)
        # nstm weights -> Wn (runs [N_START, NRUNS))
        p = pos_of[cn]
        assert p.min() >= SO_MIN and p.max() < U
        np.add.at(Wn[ch], (p % 128, p // 128 - N_START, rn, m_of), val)
        # bias: weight 1 for every batch row, both halves
        pb = n_so + pad_so + len(shared)
        Ws[ch, pb % 128, pb // 128, 0, :] = 1.0
        Wn[ch, pb % 128, pb // 128 - N_START, 0, :] = 1.0

    return {
        "t2": T2,
        "idx": idx16,
        "w_stm": Ws.reshape(CH, 128, S_RUNS * 256).astype(FP8),
        "w_nstm": Wn.reshape(CH, 128, N_RUNS * 256).astype(FP8),
        "w_sel": wsel_all[rows].reshape(CH, 128, 2 * F).astype(BF16),
        "ob_sel": obsel_all[rows].reshape(CH, 128, 1).astype(np.float32),
    }


def build_in_maps(values, stm_indices, nstm_indices, buckets, ft_w, ft_b, fft_w, fft_b, out_w, out_b):
    values = np.asarray(values, dtype=np.float32)
    stm_indices = np.asarray(stm_indices, dtype=np.int32)
    nstm_indices = np.asarray(nstm_indices, dtype=np.int32)
    buckets = np.asarray(buckets, dtype=np.int32)
    ft_w = np.asarray(ft_w, dtype=np.float32)
    ft_b = np.asarray(ft_b, dtype=np.float32)
    fft_w = np.asarray(fft_w, dtype=np.float32)
    fft_b = np.asarray(fft_b, dtype=np.float32)
    out_w = np.asarray(out_w, dtype=np.float32)
    out_b = np.asarray(out_b, dtype=np.float32)

    T = ft_w + np.tile(fft_w, (FT_VOCAB // FFT_VOCAB, 1))     # [40960, 512]
    T2 = np.zeros((CELLS + 1, 2 * F), np.float32)
    T2[:CELLS] = T.reshape(CELLS, 2 * F)                      # cell c = rows 2c, 2c+1
    T2[CELLS, :F] = ft_b + fft_b                              # bias cell, row 0
    T2 = T2.astype(FP8)

    wsel_all = out_w[buckets]                                 # [B, 1024] f32
    obsel_all = out_b[buckets]                                # [B] f32

    return [
        _prep_core(c, T2, values, stm_indices, nstm_indices, wsel_all, obsel_all)
        for c in range(N_CORES)
    ]


def kernel(**inputs):
    nc = _get_compiled()
    in_maps = build_in_maps(**inputs)
    res = run_bass_kernel_spmd(nc, in_maps, core_ids=list(range(N_CORES)))
    out = np.concatenate([res.results[c]["out"] for c in range(N_CORES)], axis=0)
    return out.astype(np.float32)



# revision 3
# speedup vs baseline: 1.3242x; 1.3242x over previous
"""NNUE HalfKP embedding-bag kernel, v2: fp8 pair-cell gather + DoubleRow matmuls.

The v1 kernel is bottlenecked by gpsimd dma_gather descriptor generation
(~7.8 ns per index, serial on the Q7 pair). v2 attacks the index count:

  * Table stored as PAIR CELLS: T2[c] = [T[2c]; T[2c+1]] in fp8 -> one gather
    index fetches TWO vocab rows (1 KB). Lookups are deduplicated at cell
    granularity per 128-row batch chunk ACROSS both stm+nstm sets:
    8192 lookups -> ~6750 distinct cells (-25% descriptors).
  * Cell index < 20481 fits int16 -> single gather per chunk (no low/high
    vocab split).
  * Routing/weighting on the PE with fp8 DoubleRow matmuls: each slot holds
    2 rows (reduction tile 2 per partition), lhsT [128, 2, 128] carries
    per-(row, batch) weights. Cells ordered [stm-only | shared | nstm-only]
    so the stm PSUM only consumes runs [0, S_RUNS) and the nstm PSUM only
    runs [N_START, NRUNS) (static windows, host-side placement asserts).
  * Each chunk gather is split into two sub-gathers (28 + 27 runs) to stay
    under the 256-descriptor/engine SWDGE ring carveout; pad slots gather
    BIAS_CELL (the trailing--1 trim path wedges the device).
  * Bucket selection folded host-side into w_sel/ob_sel as in v1.
"""

import sys

sys.path.insert(0, "/opt/trn_rl_repo")

import numpy as np
import ml_dtypes

import concourse.bass as bass
import concourse.mybir as mybir
from concourse import bacc
from concourse.tile import TileContext
from concourse.bass_utils import run_bass_kernel_spmd

FP8 = ml_dtypes.float8_e4m3
BF16 = ml_dtypes.bfloat16

B = 8192
K = 32
F = 512
FT_VOCAB = 40960
FFT_VOCAB = 640
N_CORES = 8
BC = B // N_CORES          # rows per core = 1024
CH = BC // 128             # chunks per core = 8

CELLS = FT_VOCAB // 2      # 20480 pair cells
BIAS_CELL = CELLS          # cell 20480 = [bias_row; zeros]
CAP = 7040                 # gather slot capacity per chunk (55 runs)
NRUNS = CAP // 128         # 55
S_RUNS = 32                # stm PSUM consumes runs [0, 32)
N_START = 22               # nstm PSUM consumes runs [22, 55)
N_RUNS = NRUNS - N_START   # 33
SO_MIN = N_START * 128     # stm-only region padded to >= 2816
SREF_MAX = S_RUNS * 128    # stm-referenced cells must end <= 4096
NIDX16 = CAP // 16         # 448 int16 idx columns
# The SWDGE descriptor ring carveout is 16 KB/partition -> 256 descriptors per
# DMA-engine ring; one gather may generate at most ~16*255 descriptors, so the
# 7040-slot chunk gather is split into two sub-gathers (28 + 27 runs).
GSPLIT = 3584              # 28 runs, 225 descs/engine < 256

GATH_BUFS = 2

_compiled = None


def _build():
    # 4 SWDGE queues: each dma_gather's descriptor-ring await_space (which
    # blocks until the PREVIOUS gather's DMAs drain — 225+217 descs > 256-desc
    # ring) lands on a different queue's ring, so ring-drain overlaps with the
    # next gather's descriptor generation instead of serializing inside it.
    nc = bacc.Bacc(
        "TRN2",
        target_bir_lowering=False,
        debug=False,
        num_devices=N_CORES,
        num_swdge_queues=4,
    )

    t2_d = nc.dram_tensor("t2", [CELLS + 1, 2 * F], mybir.dt.float8e4, kind="ExternalInput")
    idx_d = nc.dram_tensor("idx", [CH, 128, NIDX16], mybir.dt.int16, kind="ExternalInput")
    ws_d = nc.dram_tensor("w_stm", [CH, 128, S_RUNS * 256], mybir.dt.float8e4, kind="ExternalInput")
    wn_d = nc.dram_tensor("w_nstm", [CH, 128, N_RUNS * 256], mybir.dt.float8e4, kind="ExternalInput")
    wsel_d = nc.dram_tensor("w_sel", [CH, 128, 2 * F], mybir.dt.bfloat16, kind="ExternalInput")
    obsel_d = nc.dram_tensor("ob_sel", [CH, 128, 1], mybir.dt.float32, kind="ExternalInput")
    out_d = nc.dram_tensor("out", [BC, 1], mybir.dt.float32, kind="ExternalOutput")

    with TileContext(nc) as tc:
        with (
            tc.tile_pool(name="idx", bufs=CH) as idxp,
            tc.tile_pool(name="gath", bufs=GATH_BUFS) as gathp,
            tc.tile_pool(name="wblk", bufs=2) as wblkp,
            tc.tile_pool(name="psum", bufs=4, space="PSUM") as psump,
            tc.tile_pool(name="hid", bufs=2) as hidp,
            tc.tile_pool(name="wsel", bufs=2) as wselp,
            tc.tile_pool(name="fin", bufs=4) as finp,
        ):
            # prefetch every chunk's idx tile up front so gathers never wait
            # behind the Sync engine's per-chunk DMA queue
            idxts = []
            for ch in range(CH):
                idxt = idxp.tile([128, NIDX16], mybir.dt.int16, tag="idx")
                nc.sync.dma_start(out=idxt[:], in_=idx_d[ch])
                idxts.append(idxt)

            for ch in range(CH):
                idxt = idxts[ch]
                rt = gathp.tile([128, NRUNS * 2 * F], mybir.dt.float8e4, tag="gath")
                # sub-gather 1: positions [0, GSPLIT) — never trimmed
                nc.gpsimd.dma_gather(
                    out_ap=rt[:, : GSPLIT // 128 * 2 * F].rearrange(
                        "p (s e) -> p s e", e=2 * F
                    ),
                    in_ap=t2_d[:, :],
                    idxs_ap=idxt[:, : GSPLIT // 16],
                    num_idxs=GSPLIT,
                    num_idxs_reg=GSPLIT,
                    elem_size=2 * F,
                    single_packet=False,
                    queue_num=(2 * ch) % 4,
                )
                # sub-gather 2: positions [GSPLIT, CAP). No trailing-negative
                # trim: the trim path (value_load num_idxs_reg + -1 indices)
                # reproducibly wedges the device, so pads are valid BIAS_CELL
                # indices gathered at full count.
                nc.gpsimd.dma_gather(
                    out_ap=rt[:, GSPLIT // 128 * 2 * F :].rearrange(
                        "p (s e) -> p s e", e=2 * F
                    ),
                    in_ap=t2_d[:, :],
                    idxs_ap=idxt[:, GSPLIT // 16 :],
                    num_idxs=CAP - GSPLIT,
                    num_idxs_reg=CAP - GSPLIT,
                    elem_size=2 * F,
                    single_packet=False,
                    queue_num=(2 * ch + 1) % 4,
                )

                # W blocks ride the Scalar engine's HWDGE queue so they never
                # queue behind the Sync engine's output-stage DMAs
                wst = wblkp.tile([128, S_RUNS * 256], mybir.dt.float8e4, tag="ws")
                nc.scalar.dma_start(out=wst[:], in_=ws_d[ch])
                wnt = wblkp.tile([128, N_RUNS * 256], mybir.dt.float8e4, tag="wn")
                nc.scalar.dma_start(out=wnt[:], in_=wn_d[ch])

                hid = hidp.tile([128, 2 * F], mybir.dt.bfloat16)

                ps_s = psump.tile([128, F], mybir.dt.float32, tag="ps_s")
                for q in range(S_RUNS):
                    nc.tensor.matmul(
                        out=ps_s[:],
                        lhsT=wst[:, q * 256 : (q + 1) * 256].rearrange(
                            "p (i m) -> p i m", i=2
                        ),
                        rhs=rt[:, q * 2 * F : (q + 1) * 2 * F].rearrange(
                            "p (i n) -> p i n", i=2
                        ),
                        start=(q == 0),
                        stop=(q == S_RUNS - 1),
                        perf_mode=mybir.MatmulPerfMode.DoubleRow,
                    )
                nc.vector.tensor_scalar(
                    out=hid[:, :F],
                    in0=ps_s[:],
                    scalar1=0.0,
                    scalar2=1.0,
                    op0=mybir.AluOpType.max,
                    op1=mybir.AluOpType.min,
                )

                ps_n = psump.tile([128, F], mybir.dt.float32, tag="ps_n")
                for j in range(N_RUNS):
                    q = N_START + j
                    nc.tensor.matmul(
                        out=ps_n[:],
                        lhsT=wnt[:, j * 256 : (j + 1) * 256].rearrange(
                            "p (i m) -> p i m", i=2
                        ),
                        rhs=rt[:, q * 2 * F : (q + 1) * 2 * F].rearrange(
                            "p (i n) -> p i n", i=2
                        ),
                        start=(j == 0),
                        stop=(j == N_RUNS - 1),
                        perf_mode=mybir.MatmulPerfMode.DoubleRow,
                    )
                nc.vector.tensor_scalar(
                    out=hid[:, F:],
                    in0=ps_n[:],
                    scalar1=0.0,
                    scalar2=1.0,
                    op0=mybir.AluOpType.max,
                    op1=mybir.AluOpType.min,
                )

                wsel = wselp.tile([128, 2 * F], mybir.dt.bfloat16)
                nc.sync.dma_start(out=wsel[:], in_=wsel_d[ch])
                obsel = finp.tile([128, 1], mybir.dt.float32, tag="ob")
                nc.sync.dma_start(out=obsel[:], in_=obsel_d[ch])

                prod = finp.tile([128, 2 * F], mybir.dt.float32, tag="prod")
                nc.vector.tensor_tensor(
                    out=prod[:], in0=hid[:], in1=wsel[:], op=mybir.AluOpType.mult
                )
                acc = finp.tile([128, 1], mybir.dt.float32, tag="acc")
                nc.vector.tensor_reduce(
                    out=acc[:], in_=prod[:], axis=mybir.AxisListType.X, op=mybir.AluOpType.add
                )
                sig = finp.tile([128, 1], mybir.dt.float32, tag="sig")
                nc.scalar.activation(
                    out=sig[:],
                    in_=acc[:],
                    func=mybir.ActivationFunctionType.Sigmoid,
                    bias=obsel[:],
                )
                nc.sync.dma_start(out=out_d[ch * 128 : (ch + 1) * 128, :], in_=sig[:])

    nc.compile()
    return nc


def _get_compiled():
    global _compiled
    if _compiled is None:
        _compiled = _build()
    return _compiled


def _wrap16(lst):
    """int16 index list -> [128, len/16] wrapped (i -> [i%16, i//16]) + replicated."""
    n = lst.shape[0]
    w = lst.reshape(n // 16, 16).T.astype(np.int16)     # [16, n/16]
    return np.tile(w, (8, 1))                            # [128, n/16]


def _prep_core(core, T2, values, stm, nstm, wsel_all, obsel_all):
    rows = slice(core * BC, (core + 1) * BC)
    v_core = np.asarray(values[rows], dtype=np.float32)
    stm_c = np.asarray(stm[rows], dtype=np.int64)
    nstm_c = np.asarray(nstm[rows], dtype=np.int64)

    idx16 = np.zeros((CH, 128, NIDX16), np.int16)
    Ws = np.zeros((CH, 128, S_RUNS, 2, 128), np.float32)
    Wn = np.zeros((CH, 128, N_RUNS, 2, 128), np.float32)

    m_of = np.repeat(np.arange(128), K)                  # flat lookup -> batch row

    for ch in range(CH):
        sl = slice(ch * 128, (ch + 1) * 128)
        val = v_core[sl].reshape(-1)
        cs = (stm_c[sl].reshape(-1) // 2)
        rs = (stm_c[sl].reshape(-1) % 2)
        cn = (nstm_c[sl].reshape(-1) // 2)
        rn = (nstm_c[sl].reshape(-1) % 2)

        u_s = np.unique(cs)
        u_n = np.unique(cn)
        shared = np.intersect1d(u_s, u_n, assume_unique=True)
        s_only = np.setdiff1d(u_s, shared, assume_unique=True)
        n_only = np.setdiff1d(u_n, shared, assume_unique=True)

        n_so = len(s_only)
        pad_so = max(0, SO_MIN - n_so)                   # keep nstm window start static
        layout = np.concatenate([
            s_only,
            # dummy pad: BIAS_CELL can never alias a real lookup (cells from
            # real indices are < CELLS), so pos_of stays unambiguous
            np.full(pad_so, BIAS_CELL, np.int64),
            shared,
            np.array([BIAS_CELL], np.int64),
            n_only,
        ])
        U = len(layout)
        sref_end = n_so + pad_so + len(shared) + 1
        assert sref_end <= SREF_MAX, (n_so, pad_so, len(shared))
        assert U <= CAP, U

        pos_of = np.empty(CELLS + 1, np.int32)
        # duplicates from pad cells are fine: only the FIRST occurrence
        # position is referenced by weights (assign in reverse so first wins)
        pos_of[layout[::-1]] = np.arange(U - 1, -1, -1, dtype=np.int32)

        ilist = np.full(CAP, BIAS_CELL, np.int16)   # tail pad: gathered, weight 0
        ilist[:U] = layout
        assert U > GSPLIT, U
        idx16[ch] = _wrap16(ilist)

        # stm weights -> Ws (runs [0, S_RUNS))
        p = pos_of[cs]
        assert p.max() < SREF_MAX
        np.add.at(Ws[ch], (p % 128, p // 128, rs, m_of), val)
        # nstm weights -> Wn (runs [N_START, NRUNS))
        p = pos_of[cn]
        assert p.min() >= SO_MIN and p.max() < U
        np.add.at(Wn[ch], (p % 128, p // 128 - N_START, rn, m_of), val)
        # bias: weight 1 for every batch row, both halves
        pb = n_so + pad_so + len(shared)
        Ws[ch, pb % 128, pb // 128, 0, :] = 1.0
        Wn[ch, pb % 128, pb // 128 - N_START, 0, :] = 1.0

    return {
        "t2": T2,
        "idx": idx16,
        "w_stm": Ws.reshape(CH, 128, S_RUNS * 256).astype(FP8),
        "w_nstm": Wn.reshape(CH, 128, N_RUNS * 256).astype(FP8),
        "w_sel": wsel_all[rows].reshape(CH, 128, 2 * F).astype(BF16),
        "ob_sel": obsel_all[rows].reshape(CH, 128, 1).astype(np.float32),
    }


def build_in_maps(values, stm_indices, nstm_indices, buckets, ft_w, ft_b, fft_w, fft_b, out_w, out_b):
    values = np.asarray(values, dtype=np.float32)
    stm_indices = np.asarray(stm_indices, dtype=np.int32)
    nstm_indices = np.asarray(nstm_indices, dtype=np.int32)
    buckets = np.asarray(buckets, dtype=np.int32)
    ft_w = np.asarray(ft_w, dtype=np.float32)
    ft_b = np.asarray(ft_b, dtype=np.float32)
    fft_w = np.asarray(fft_w, dtype=np.float32)
    fft_b = np.asarray(fft_b, dtype=np.float32)
    out_w = np.asarray(out_w, dtype=np.float32)
    out_b = np.asarray(out_b, dtype=np.float32)

    T = ft_w + np.tile(fft_w, (FT_VOCAB // FFT_VOCAB, 1))     # [40960, 512]
    T2 = np.zeros((CELLS + 1, 2 * F), np.float32)
    T2[:CELLS] = T.reshape(CELLS, 2 * F)                      # cell c = rows 2c, 2c+1
    T2[CELLS, :F] = ft_b + fft_b                              # bias cell, row 0
    T2 = T2.astype(FP8)

    wsel_all = out_w[buckets]                                 # [B, 1024] f32
    obsel_all = out_b[buckets]                                # [B] f32

    return [
        _prep_core(c, T2, values, stm_indices, nstm_indices, wsel_all, obsel_all)
        for c in range(N_CORES)
    ]


def kernel(**inputs):
    nc = _get_compiled()
    in_maps = build_in_maps(**inputs)
    res = run_bass_kernel_spmd(nc, in_maps, core_ids=list(range(N_CORES)))
    out = np.concatenate([res.results[c]["out"] for c in range(N_CORES)], axis=0)
    return out.astype(np.float32)



# revision 5
# speedup vs baseline: 1.8356x; 1.3862x over previous
"""NNUE HalfKP embedding-bag kernel, v4: parity-split row gathers + 4 SWDGE queues.

v2 gathered 1 KB PAIR cells (both rows of a cell) to halve descriptor count,
but ~91% of gathered pair cells use only one row: ~45% of gather DMA traffic
and ~50% of the lhsT weight block were structural waste (DoubleRow dead rows).

v4 keeps cell-granular int16 indices (cell id < 20481) but fetches single
512 B ROWS via two gathers per chunk with elem_step=1024:

  * even gather: in_ap = T2[:, :512]  -> row 2c for index c
  * odd  gather: in_ap = T2[:, 512:]  -> row 2c+1 for index c

  Lookups are deduplicated at (cell, parity) granularity per 128-row batch
  chunk across stm+nstm: ~3715 distinct cells per chunk-parity -> one gather
  per parity (3968 idx = 249 descs/engine, fits the 256-desc ring).

  * Matmuls are plain fp8 [128k,128m]x[128k,512n] (no DoubleRow -> FWL fast
    weight loads). Cells ordered [stm-only | pad | shared (+bias, even) |
    nstm-only]; stm PSUM consumes runs [0,17) of both parities, nstm PSUM
    runs [14,31) (static windows, host-side placement asserts).
  * 4 SWDGE queues, round-robin: each dma_gather's ring await_space lands on
    its own queue ring, and the 4 Q7 core pairs generate descriptors
    CONCURRENTLY (the NX instruction queue lets idle pairs run ahead).
  * Bucket selection folded host-side into w_sel/ob_sel.
"""

import sys

sys.path.insert(0, "/opt/trn_rl_repo")

import numpy as np
import ml_dtypes

import concourse.bass as bass
import concourse.mybir as mybir
from concourse import bacc
from concourse.tile import TileContext
from concourse.bass_utils import run_bass_kernel_spmd

FP8 = ml_dtypes.float8_e4m3
BF16 = ml_dtypes.bfloat16

B = 8192
K = 32
F = 512
FT_VOCAB = 40960
FFT_VOCAB = 640
N_CORES = 8
BC = B // N_CORES          # rows per core = 1024
CH = BC // 128             # chunks per core = 8

CELLS = FT_VOCAB // 2      # 20480 pair cells (rows 2c, 2c+1)
BIAS_CELL = CELLS          # cell 20480: row0 = bias, row1 = zeros
S_RUNS_P = 17              # stm window runs per parity (2176 slots)
N_START_P = 14             # nstm window start run (slot 1792)
NRUNS_P = 31               # gather slots per parity = 3968 (249 descs < 256)
N_RUNS_P = NRUNS_P - N_START_P  # 17
CAP_P = NRUNS_P * 128      # 3968
SO_MIN_P = N_START_P * 128 # stm-only region padded to >= 1792
SREF_MAX_P = S_RUNS_P * 128
NIDX_P = CAP_P // 16       # 248 int16 idx columns per parity

GATH_BUFS = 5              # 2.5 chunks of (even, odd) gather tiles in flight

_compiled = None


def _build():
    # 4 SWDGE queues: each dma_gather's descriptor-ring await_space (which
    # blocks until the ring's previous gather DMAs drain) lands on its own
    # queue, and the 4 Q7 pairs generate descriptors concurrently.
    nc = bacc.Bacc(
        "TRN2",
        target_bir_lowering=False,
        debug=False,
        num_devices=N_CORES,
        num_swdge_queues=4,
    )

    t2_d = nc.dram_tensor("t2", [CELLS + 1, 2 * F], mybir.dt.float8e4, kind="ExternalInput")
    idx_d = nc.dram_tensor("idx", [CH, 128, 2 * NIDX_P], mybir.dt.int16, kind="ExternalInput")
    ws_d = nc.dram_tensor("w_stm", [CH, 128, 2 * S_RUNS_P * 128], mybir.dt.float8e4, kind="ExternalInput")
    wn_d = nc.dram_tensor("w_nstm", [CH, 128, 2 * N_RUNS_P * 128], mybir.dt.float8e4, kind="ExternalInput")
    wsel_d = nc.dram_tensor("w_sel", [CH, 128, 2 * F], mybir.dt.bfloat16, kind="ExternalInput")
    obsel_d = nc.dram_tensor("ob_sel", [CH, 128, 1], mybir.dt.float32, kind="ExternalInput")
    out_d = nc.dram_tensor("out", [BC, 1], mybir.dt.float32, kind="ExternalOutput")

    with TileContext(nc) as tc:
        with (
            tc.tile_pool(name="idx", bufs=CH) as idxp,
            tc.tile_pool(name="gath", bufs=GATH_BUFS) as gathp,
            tc.tile_pool(name="wblk", bufs=2) as wblkp,
            tc.tile_pool(name="psum", bufs=4, space="PSUM") as psump,
            tc.tile_pool(name="hid", bufs=2) as hidp,
            tc.tile_pool(name="wsel", bufs=2) as wselp,
            tc.tile_pool(name="fin", bufs=4) as finp,
        ):
            # prefetch every chunk's idx tile up front so gathers never wait
            # behind the Sync engine's per-chunk DMA queue
            idxts = []
            for ch in range(CH):
                idxt = idxp.tile([128, 2 * NIDX_P], mybir.dt.int16, tag="idx")
                nc.sync.dma_start(out=idxt[:], in_=idx_d[ch])
                idxts.append(idxt)

            for ch in range(CH):
                idxt = idxts[ch]
                rt_e = gathp.tile([128, NRUNS_P * F], mybir.dt.float8e4, tag="gath_e")
                rt_o = gathp.tile([128, NRUNS_P * F], mybir.dt.float8e4, tag="gath_o")
                # even rows (2c): base T2[:, :512]; odd rows (2c+1): T2[:, 512:].
                # elem_step=1024 elems strides whole pair cells. Pad slots
                # gather BIAS_CELL (valid; the trailing--1 trim path wedges
                # the device) and carry zero weight.
                nc.gpsimd.dma_gather(
                    out_ap=rt_e[:].rearrange("p (s e) -> p s e", e=F),
                    in_ap=t2_d[:, :F],
                    idxs_ap=idxt[:, :NIDX_P],
                    num_idxs=CAP_P,
                    num_idxs_reg=CAP_P,
                    elem_size=F,
                    elem_step=2 * F,
                    single_packet=False,
                    queue_num=(2 * ch) % 4,
                )
                nc.gpsimd.dma_gather(
                    out_ap=rt_o[:].rearrange("p (s e) -> p s e", e=F),
                    in_ap=t2_d[:, F:],
                    idxs_ap=idxt[:, NIDX_P:],
                    num_idxs=CAP_P,
                    num_idxs_reg=CAP_P,
                    elem_size=F,
                    elem_step=2 * F,
                    single_packet=False,
                    queue_num=(2 * ch + 1) % 4,
                )

                # W blocks ride the Scalar engine's HWDGE queue so they never
                # queue behind the Sync engine's output-stage DMAs
                wst = wblkp.tile([128, 2 * S_RUNS_P * 128], mybir.dt.float8e4, tag="ws")
                nc.scalar.dma_start(out=wst[:], in_=ws_d[ch])
                wnt = wblkp.tile([128, 2 * N_RUNS_P * 128], mybir.dt.float8e4, tag="wn")
                nc.scalar.dma_start(out=wnt[:], in_=wn_d[ch])

                hid = hidp.tile([128, 2 * F], mybir.dt.bfloat16)

                ps_s = psump.tile([128, F], mybir.dt.float32, tag="ps_s")
                ps_n = psump.tile([128, F], mybir.dt.float32, tag="ps_n")
                # even-parity matmuls first (both PSUMs), so the PE starts as
                # soon as rt_e lands even if rt_o is still gathering.
                for par, rt in ((0, rt_e), (1, rt_o)):
                    for q in range(S_RUNS_P):
                        nc.tensor.matmul(
                            out=ps_s[:],
                            lhsT=wst[:, (par * S_RUNS_P + q) * 128 : (par * S_RUNS_P + q + 1) * 128],
                            rhs=rt[:, q * F : (q + 1) * F],
                            start=(par == 0 and q == 0),
                            stop=(par == 1 and q == S_RUNS_P - 1),
                        )
                    for j in range(N_RUNS_P):
                        q = N_START_P + j
                        nc.tensor.matmul(
                            out=ps_n[:],
                            lhsT=wnt[:, (par * N_RUNS_P + j) * 128 : (par * N_RUNS_P + j + 1) * 128],
                            rhs=rt[:, q * F : (q + 1) * F],
                            start=(par == 0 and j == 0),
                            stop=(par == 1 and j == N_RUNS_P - 1),
                        )
                nc.vector.tensor_scalar(
                    out=hid[:, :F],
                    in0=ps_s[:],
                    scalar1=0.0,
                    scalar2=1.0,
                    op0=mybir.AluOpType.max,
                    op1=mybir.AluOpType.min,
                )
                nc.vector.tensor_scalar(
                    out=hid[:, F:],
                    in0=ps_n[:],
                    scalar1=0.0,
                    scalar2=1.0,
                    op0=mybir.AluOpType.max,
                    op1=mybir.AluOpType.min,
                )

                wsel = wselp.tile([128, 2 * F], mybir.dt.bfloat16)
                nc.sync.dma_start(out=wsel[:], in_=wsel_d[ch])
                obsel = finp.tile([128, 1], mybir.dt.float32, tag="ob")
                nc.sync.dma_start(out=obsel[:], in_=obsel_d[ch])

                prod = finp.tile([128, 2 * F], mybir.dt.float32, tag="prod")
                nc.vector.tensor_tensor(
                    out=prod[:], in0=hid[:], in1=wsel[:], op=mybir.AluOpType.mult
                )
                acc = finp.tile([128, 1], mybir.dt.float32, tag="acc")
                nc.vector.tensor_reduce(
                    out=acc[:], in_=prod[:], axis=mybir.AxisListType.X, op=mybir.AluOpType.add
                )
                sig = finp.tile([128, 1], mybir.dt.float32, tag="sig")
                nc.scalar.activation(
                    out=sig[:],
                    in_=acc[:],
                    func=mybir.ActivationFunctionType.Sigmoid,
                    bias=obsel[:],
                )
                nc.sync.dma_start(out=out_d[ch * 128 : (ch + 1) * 128, :], in_=sig[:])

    nc.compile()
    return nc


def _get_compiled():
    global _compiled
    if _compiled is None:
        _compiled = _build()
    return _compiled


def _wrap16(lst):
    """int16 index list -> [128, len/16] wrapped (i -> [i%16, i//16]) + replicated."""
    n = lst.shape[0]
    w = lst.reshape(n // 16, 16).T.astype(np.int16)     # [16, n/16]
    return np.tile(w, (8, 1))                            # [128, n/16]


def _prep_core(core, T2, values, stm, nstm, wsel_all, obsel_all):
    rows = slice(core * BC, (core + 1) * BC)
    v_core = np.asarray(values[rows], dtype=np.float32)
    stm_c = np.asarray(stm[rows], dtype=np.int64)
    nstm_c = np.asarray(nstm[rows], dtype=np.int64)

    idx16 = np.zeros((CH, 128, 2 * NIDX_P), np.int16)
    Ws = np.zeros((CH, 128, 2 * S_RUNS_P, 128), np.float32)
    Wn = np.zeros((CH, 128, 2 * N_RUNS_P, 128), np.float32)

    m_of = np.repeat(np.arange(128), K)                  # flat lookup -> batch row

    for ch in range(CH):
        sl = slice(ch * 128, (ch + 1) * 128)
        val = v_core[sl].reshape(-1)
        rs_all = stm_c[sl].reshape(-1)
        rn_all = nstm_c[sl].reshape(-1)

        for par in range(2):
            ms = rs_all % 2 == par
            mn = rn_all % 2 == par
            cs = rs_all[ms] // 2
            cn = rn_all[mn] // 2

            u_s = np.unique(cs)
            u_n = np.unique(cn)
            shared = np.intersect1d(u_s, u_n, assume_unique=True)
            s_only = np.setdiff1d(u_s, shared, assume_unique=True)
            n_only = np.setdiff1d(u_n, shared, assume_unique=True)

            n_so = len(s_only)
            pad_so = max(0, SO_MIN_P - n_so)             # keep nstm window start static
            parts = [
                s_only,
                # dummy pad: BIAS_CELL can never alias a real lookup, so
                # pos_of stays unambiguous (first occurrence wins)
                np.full(pad_so, BIAS_CELL, np.int64),
                shared,
            ]
            if par == 0:
                parts.append(np.array([BIAS_CELL], np.int64))   # bias slot
            parts.append(n_only)
            layout = np.concatenate(parts)
            U = len(layout)
            sref_end = n_so + pad_so + len(shared) + (1 if par == 0 else 0)
            assert sref_end <= SREF_MAX_P, (n_so, pad_so, len(shared))
            assert U <= CAP_P, U

            pos_of = np.empty(CELLS + 1, np.int32)
            pos_of[layout[::-1]] = np.arange(U - 1, -1, -1, dtype=np.int32)

            ilist = np.full(CAP_P, BIAS_CELL, np.int16)  # tail pad: gathered, weight 0
            ilist[:U] = layout
            idx16[ch, :, par * NIDX_P : (par + 1) * NIDX_P] = _wrap16(ilist)

            # stm weights -> Ws runs [par*S_RUNS_P, par*S_RUNS_P + S_RUNS_P)
            p = pos_of[cs]
            assert p.max() < SREF_MAX_P
            np.add.at(Ws[ch], (p % 128, par * S_RUNS_P + p // 128, m_of[ms]), val[ms])
            # nstm weights -> Wn runs [par*N_RUNS_P, ...)
            p = pos_of[cn]
            assert p.min() >= SO_MIN_P and p.max() < U
            np.add.at(
                Wn[ch],
                (p % 128, par * N_RUNS_P + p // 128 - N_START_P, m_of[mn]),
                val[mn],
            )
            if par == 0:
                # bias: weight 1 for every batch row, in both PSUM windows
                pb = n_so + pad_so + len(shared)
                assert pb >= SO_MIN_P
                Ws[ch, pb % 128, pb // 128, :] = 1.0
                Wn[ch, pb % 128, pb // 128 - N_START_P, :] = 1.0

    return {
        "t2": T2,
        "idx": idx16,
        "w_stm": Ws.reshape(CH, 128, 2 * S_RUNS_P * 128).astype(FP8),
        "w_nstm": Wn.reshape(CH, 128, 2 * N_RUNS_P * 128).astype(FP8),
        "w_sel": wsel_all[rows].reshape(CH, 128, 2 * F).astype(BF16),
        "ob_sel": obsel_all[rows].reshape(CH, 128, 1).astype(np.float32),
    }


def build_in_maps(values, stm_indices, nstm_indices, buckets, ft_w, ft_b, fft_w, fft_b, out_w, out_b):
    values = np.asarray(values, dtype=np.float32)
    stm_indices = np.asarray(stm_indices, dtype=np.int32)
    nstm_indices = np.asarray(nstm_indices, dtype=np.int32)
    buckets = np.asarray(buckets, dtype=np.int32)
    ft_w = np.asarray(ft_w, dtype=np.float32)
    ft_b = np.asarray(ft_b, dtype=np.float32)
    fft_w = np.asarray(fft_w, dtype=np.float32)
    fft_b = np.asarray(fft_b, dtype=np.float32)
    out_w = np.asarray(out_w, dtype=np.float32)
    out_b = np.asarray(out_b, dtype=np.float32)

    T = ft_w + np.tile(fft_w, (FT_VOCAB // FFT_VOCAB, 1))     # [40960, 512]
    T2 = np.zeros((CELLS + 1, 2 * F), np.float32)
    T2[:CELLS] = T.reshape(CELLS, 2 * F)                      # cell c = rows 2c, 2c+1
    T2[CELLS, :F] = ft_b + fft_b                              # bias cell, row 0
    T2 = T2.astype(FP8)

    wsel_all = out_w[buckets]                                 # [B, 1024] f32
    obsel_all = out_b[buckets]                                # [B] f32

    return [
        _prep_core(c, T2, values, stm_indices, nstm_indices, wsel_all, obsel_all)
        for c in range(N_CORES)
    ]


def kernel(**inputs):
    nc = _get_compiled()
    in_maps = build_in_maps(**inputs)
    res = run_bass_kernel_spmd(nc, in_maps, core_ids=list(range(N_CORES)))
    out = np.concatenate([res.results[c]["out"] for c in range(N_CORES)], axis=0)
    return out.astype(np.float32)


# revision 9
# speedup vs baseline: 1.9795x; 1.0784x over previous
"""NNUE HalfKP embedding-bag kernel, v4: parity-split row gathers + 4 SWDGE queues.

v2 gathered 1 KB PAIR cells (both rows of a cell) to halve descriptor count,
but ~91% of gathered pair cells use only one row: ~45% of gather DMA traffic
and ~50% of the lhsT weight block were structural waste (DoubleRow dead rows).

v4 keeps cell-granular int16 indices (cell id < 20481) but fetches single
512 B ROWS via two gathers per chunk with elem_step=1024:

  * even gather: in_ap = T2[:, :512]  -> row 2c for index c
  * odd  gather: in_ap = T2[:, 512:]  -> row 2c+1 for index c

  Lookups are deduplicated at (cell, parity) granularity per 128-row batch
  chunk across stm+nstm: ~3715 distinct cells per chunk-parity -> one gather
  per parity (3968 idx = 249 descs/engine, fits the 256-desc ring).

  * Matmuls are plain fp8 [128k,128m]x[128k,512n] (no DoubleRow -> FWL fast
    weight loads). Cells ordered [stm-only | pad | shared (+bias, even) |
    nstm-only]; stm PSUM consumes runs [0,17) of both parities, nstm PSUM
    runs [14,31) (static windows, host-side placement asserts).
  * 4 SWDGE queues, round-robin: each dma_gather's ring await_space lands on
    its own queue ring, and the 4 Q7 core pairs generate descriptors
    CONCURRENTLY (the NX instruction queue lets idle pairs run ahead).
  * Bucket selection folded host-side into w_sel/ob_sel.
"""

import sys

sys.path.insert(0, "/opt/trn_rl_repo")

import numpy as np
import ml_dtypes

import concourse.bass as bass
import concourse.mybir as mybir
from concourse import bacc
from concourse.tile import TileContext
from concourse.bass_utils import run_bass_kernel_spmd

FP8 = ml_dtypes.float8_e4m3
BF16 = ml_dtypes.bfloat16

B = 8192
K = 32
F = 512
FT_VOCAB = 40960
FFT_VOCAB = 640
N_CORES = 8
BC = B // N_CORES          # rows per core = 1024
CH = BC // 128             # chunks per core = 8

CELLS = FT_VOCAB // 2      # 20480 pair cells (rows 2c, 2c+1)
BIAS_CELL = CELLS          # cell 20480: row0 = bias, row1 = zeros
S_RUNS_P = 16              # stm window runs per parity (2048 slots; max sref_end=2018)
N_START_P = 14             # nstm window start run (slot 1792)
NRUNS_P = 30               # gather slots per parity = 3840 (241 descs < 256; max U=3819)
N_RUNS_P = NRUNS_P - N_START_P  # 17
CAP_P = NRUNS_P * 128      # 3968
SO_MIN_P = N_START_P * 128 # stm-only region padded to >= 1792
SREF_MAX_P = S_RUNS_P * 128
NIDX_P = CAP_P // 16       # 248 int16 idx columns per parity

GATH_BUFS = 5              # 5 chunks of (even, odd) gather tile pairs in flight

_compiled = None


def _build():
    # 4 SWDGE queues: each dma_gather's descriptor-ring await_space (which
    # blocks until the ring's previous gather DMAs drain) lands on its own
    # queue, and the 4 Q7 pairs generate descriptors concurrently.
    nc = bacc.Bacc(
        "TRN2",
        target_bir_lowering=False,
        debug=False,
        num_devices=N_CORES,
        num_swdge_queues=4,
    )

    t2_d = nc.dram_tensor("t2", [CELLS + 1, 2 * F], mybir.dt.float8e4, kind="ExternalInput")
    idx_d = nc.dram_tensor("idx", [CH, 128, 2 * NIDX_P], mybir.dt.int16, kind="ExternalInput")
    ws_d = nc.dram_tensor("w_stm", [CH, 128, 2 * S_RUNS_P * 128], mybir.dt.float8e4, kind="ExternalInput")
    wn_d = nc.dram_tensor("w_nstm", [CH, 128, 2 * N_RUNS_P * 128], mybir.dt.float8e4, kind="ExternalInput")
    wsel_d = nc.dram_tensor("w_sel", [CH, 128, 2 * F], mybir.dt.bfloat16, kind="ExternalInput")
    obsel_d = nc.dram_tensor("ob_sel", [CH, 128, 1], mybir.dt.float32, kind="ExternalInput")
    out_d = nc.dram_tensor("out", [BC, 1], mybir.dt.float32, kind="ExternalOutput")

    with TileContext(nc) as tc:
        with (
            tc.tile_pool(name="idx", bufs=CH) as idxp,
            tc.tile_pool(name="gath", bufs=GATH_BUFS) as gathp,
            tc.tile_pool(name="wblk", bufs=2) as wblkp,
            tc.tile_pool(name="psum", bufs=4, space="PSUM") as psump,
            tc.tile_pool(name="hid", bufs=2) as hidp,
            tc.tile_pool(name="wsel", bufs=2) as wselp,
            tc.tile_pool(name="fin", bufs=3) as finp,
        ):
            # prefetch every chunk's idx tile up front so gathers never wait
            # behind the Sync engine's per-chunk DMA queue
            idxts = []
            for ch in range(CH):
                idxt = idxp.tile([128, 2 * NIDX_P], mybir.dt.int16, tag="idx")
                nc.sync.dma_start(out=idxt[:], in_=idx_d[ch])
                idxts.append(idxt)

            for ch in range(CH):
                idxt = idxts[ch]
                rt_e = gathp.tile([128, NRUNS_P * F], mybir.dt.float8e4, tag="gath_e")
                rt_o = gathp.tile([128, NRUNS_P * F], mybir.dt.float8e4, tag="gath_o")
                # even rows (2c): base T2[:, :512]; odd rows (2c+1): T2[:, 512:].
                # elem_step=1024 elems strides whole pair cells. Pad slots
                # gather BIAS_CELL (valid; the trailing--1 trim path wedges
                # the device) and carry zero weight.
                nc.gpsimd.dma_gather(
                    out_ap=rt_e[:].rearrange("p (s e) -> p s e", e=F),
                    in_ap=t2_d[:, :F],
                    idxs_ap=idxt[:, :NIDX_P],
                    num_idxs=CAP_P,
                    num_idxs_reg=CAP_P,
                    elem_size=F,
                    elem_step=2 * F,
                    single_packet=False,
                    queue_num=(2 * ch) % 4,
                )
                nc.gpsimd.dma_gather(
                    out_ap=rt_o[:].rearrange("p (s e) -> p s e", e=F),
                    in_ap=t2_d[:, F:],
                    idxs_ap=idxt[:, NIDX_P:],
                    num_idxs=CAP_P,
                    num_idxs_reg=CAP_P,
                    elem_size=F,
                    elem_step=2 * F,
                    single_packet=False,
                    queue_num=(2 * ch + 1) % 4,
                )

                # W blocks ride the Scalar engine's HWDGE queue so they never
                # queue behind the Sync engine's output-stage DMAs
                wst = wblkp.tile([128, 2 * S_RUNS_P * 128], mybir.dt.float8e4, tag="ws")
                nc.scalar.dma_start(out=wst[:], in_=ws_d[ch])
                wnt = wblkp.tile([128, 2 * N_RUNS_P * 128], mybir.dt.float8e4, tag="wn")
                nc.scalar.dma_start(out=wnt[:], in_=wn_d[ch])

                hid = hidp.tile([128, 2 * F], mybir.dt.bfloat16)

                ps_s = psump.tile([128, F], mybir.dt.float32, tag="ps_s")
                ps_n = psump.tile([128, F], mybir.dt.float32, tag="ps_n")
                # even-parity matmuls first (both PSUMs), so the PE starts as
                # soon as rt_e lands even if rt_o is still gathering.
                for par, rt in ((0, rt_e), (1, rt_o)):
                    for q in range(S_RUNS_P):
                        nc.tensor.matmul(
                            out=ps_s[:],
                            lhsT=wst[:, (par * S_RUNS_P + q) * 128 : (par * S_RUNS_P + q + 1) * 128],
                            rhs=rt[:, q * F : (q + 1) * F],
                            start=(par == 0 and q == 0),
                            stop=(par == 1 and q == S_RUNS_P - 1),
                        )
                    for j in range(N_RUNS_P):
                        q = N_START_P + j
                        nc.tensor.matmul(
                            out=ps_n[:],
                            lhsT=wnt[:, (par * N_RUNS_P + j) * 128 : (par * N_RUNS_P + j + 1) * 128],
                            rhs=rt[:, q * F : (q + 1) * F],
                            start=(par == 0 and j == 0),
                            stop=(par == 1 and j == N_RUNS_P - 1),
                        )
                nc.vector.tensor_scalar(
                    out=hid[:, :F],
                    in0=ps_s[:],
                    scalar1=0.0,
                    scalar2=1.0,
                    op0=mybir.AluOpType.max,
                    op1=mybir.AluOpType.min,
                )
                nc.vector.tensor_scalar(
                    out=hid[:, F:],
                    in0=ps_n[:],
                    scalar1=0.0,
                    scalar2=1.0,
                    op0=mybir.AluOpType.max,
                    op1=mybir.AluOpType.min,
                )

                wsel = wselp.tile([128, 2 * F], mybir.dt.bfloat16)
                nc.sync.dma_start(out=wsel[:], in_=wsel_d[ch])
                obsel = finp.tile([128, 1], mybir.dt.float32, tag="ob")
                nc.sync.dma_start(out=obsel[:], in_=obsel_d[ch])

                prod = finp.tile([128, 2 * F], mybir.dt.float32, tag="prod")
                nc.vector.tensor_tensor(
                    out=prod[:], in0=hid[:], in1=wsel[:], op=mybir.AluOpType.mult
                )
                acc = finp.tile([128, 1], mybir.dt.float32, tag="acc")
                nc.vector.tensor_reduce(
                    out=acc[:], in_=prod[:], axis=mybir.AxisListType.X, op=mybir.AluOpType.add
                )
                sig = finp.tile([128, 1], mybir.dt.float32, tag="sig")
                nc.scalar.activation(
                    out=sig[:],
                    in_=acc[:],
                    func=mybir.ActivationFunctionType.Sigmoid,
                    bias=obsel[:],
                )
                nc.sync.dma_start(out=out_d[ch * 128 : (ch + 1) * 128, :], in_=sig[:])

    nc.compile()
    return nc


def _get_compiled():
    global _compiled
    if _compiled is None:
        _compiled = _build()
    return _compiled


def _wrap16(lst):
    """int16 index list -> [128, len/16] wrapped (i -> [i%16, i//16]) + replicated."""
    n = lst.shape[0]
    w = lst.reshape(n // 16, 16).T.astype(np.int16)     # [16, n/16]
    return np.tile(w, (8, 1))                            # [128, n/16]


def _prep_core(core, T2, values, stm, nstm, wsel_all, obsel_all):
    rows = slice(core * BC, (core + 1) * BC)
    v_core = np.asarray(values[rows], dtype=np.float32)
    stm_c = np.asarray(stm[rows], dtype=np.int64)
    nstm_c = np.asarray(nstm[rows], dtype=np.int64)

    idx16 = np.zeros((CH, 128, 2 * NIDX_P), np.int16)
    Ws = np.zeros((CH, 128, 2 * S_RUNS_P, 128), np.float32)
    Wn = np.zeros((CH, 128, 2 * N_RUNS_P, 128), np.float32)

    m_of = np.repeat(np.arange(128), K)                  # flat lookup -> batch row

    for ch in range(CH):
        sl = slice(ch * 128, (ch + 1) * 128)
        val = v_core[sl].reshape(-1)
        rs_all = stm_c[sl].reshape(-1)
        rn_all = nstm_c[sl].reshape(-1)

        for par in range(2):
            ms = rs_all % 2 == par
            mn = rn_all % 2 == par
            cs = rs_all[ms] // 2
            cn = rn_all[mn] // 2

            u_s = np.unique(cs)
            u_n = np.unique(cn)
            shared = np.intersect1d(u_s, u_n, assume_unique=True)
            s_only = np.setdiff1d(u_s, shared, assume_unique=True)
            n_only = np.setdiff1d(u_n, shared, assume_unique=True)

            n_so = len(s_only)
            pad_so = max(0, SO_MIN_P - n_so)             # keep nstm window start static
            parts = [
                s_only,
                # dummy pad: BIAS_CELL can never alias a real lookup, so
                # pos_of stays unambiguous (first occurrence wins)
                np.full(pad_so, BIAS_CELL, np.int64),
                shared,
            ]
            if par == 0:
                parts.append(np.array([BIAS_CELL], np.int64))   # bias slot
            parts.append(n_only)
            layout = np.concatenate(parts)
            U = len(layout)
            sref_end = n_so + pad_so + len(shared) + (1 if par == 0 else 0)
            assert sref_end <= SREF_MAX_P, (n_so, pad_so, len(shared))
            assert U <= CAP_P, U

            pos_of = np.empty(CELLS + 1, np.int32)
            pos_of[layout[::-1]] = np.arange(U - 1, -1, -1, dtype=np.int32)

            ilist = np.full(CAP_P, BIAS_CELL, np.int16)  # tail pad: gathered, weight 0
            ilist[:U] = layout
            idx16[ch, :, par * NIDX_P : (par + 1) * NIDX_P] = _wrap16(ilist)

            # stm weights -> Ws runs [par*S_RUNS_P, par*S_RUNS_P + S_RUNS_P)
            p = pos_of[cs]
            assert p.max() < SREF_MAX_P
            np.add.at(Ws[ch], (p % 128, par * S_RUNS_P + p // 128, m_of[ms]), val[ms])
            # nstm weights -> Wn runs [par*N_RUNS_P, ...)
            p = pos_of[cn]
            assert p.min() >= SO_MIN_P and p.max() < U
            np.add.at(
                Wn[ch],
                (p % 128, par * N_RUNS_P + p // 128 - N_START_P, m_of[mn]),
                val[mn],
            )
            if par == 0:
                # bias: weight 1 for every batch row, in both PSUM windows
                pb = n_so + pad_so + len(shared)
                assert pb >= SO_MIN_P
                Ws[ch, pb % 128, pb // 128, :] = 1.0
                Wn[ch, pb % 128, pb // 128 - N_START_P, :] = 1.0

    return {
        "t2": T2,
        "idx": idx16,
        "w_stm": Ws.reshape(CH, 128, 2 * S_RUNS_P * 128).astype(FP8),
        "w_nstm": Wn.reshape(CH, 128, 2 * N_RUNS_P * 128).astype(FP8),
        "w_sel": wsel_all[rows].reshape(CH, 128, 2 * F).astype(BF16),
        "ob_sel": obsel_all[rows].reshape(CH, 128, 1).astype(np.float32),
    }


def build_in_maps(values, stm_indices, nstm_indices, buckets, ft_w, ft_b, fft_w, fft_b, out_w, out_b):
    values = np.asarray(values, dtype=np.float32)
    stm_indices = np.asarray(stm_indices, dtype=np.int32)
    nstm_indices = np.asarray(nstm_indices, dtype=np.int32)
    buckets = np.asarray(buckets, dtype=np.int32)
    ft_w = np.asarray(ft_w, dtype=np.float32)
    ft_b = np.asarray(ft_b, dtype=np.float32)
    fft_w = np.asarray(fft_w, dtype=np.float32)
    fft_b = np.asarray(fft_b, dtype=np.float32)
    out_w = np.asarray(out_w, dtype=np.float32)
    out_b = np.asarray(out_b, dtype=np.float32)

    T = ft_w + np.tile(fft_w, (FT_VOCAB // FFT_VOCAB, 1))     # [40960, 512]
    T2 = np.zeros((CELLS + 1, 2 * F), np.float32)
    T2[:CELLS] = T.reshape(CELLS, 2 * F)                      # cell c = rows 2c, 2c+1
    T2[CELLS, :F] = ft_b + fft_b                              # bias cell, row 0
    T2 = T2.astype(FP8)

    wsel_all = out_w[buckets]                                 # [B, 1024] f32
    obsel_all = out_b[buckets]                                # [B] f32

    return [
        _prep_core(c, T2, values, stm_indices, nstm_indices, wsel_all, obsel_all)
        for c in range(N_CORES)
    ]


def kernel(**inputs):
    nc = _get_compiled()
    in_maps = build_in_maps(**inputs)
    res = run_bass_kernel_spmd(nc, in_maps, core_ids=list(range(N_CORES)))
    out = np.concatenate([res.results[c]["out"] for c in range(N_CORES)], axis=0)
    return out.astype(np.float32)
